# revision 7
# baseline (speedup 1.0000x reference)
"""BitLinear forward (RMSNorm -> int8 activation quant -> ternary weight quant
-> matmul -> rescale) on 8 Trainium2 NeuronCores.

Sharding: data-parallel over rows. x (4,4096,1024) flattens to (16384,1024);
each core gets 2048 rows and the full weight (4096,1024), so w_scale=mean|w|
is computed locally with no collective. Output (16384,4096) is concatenated
on the host and reshaped to (4,4096,4096).

Math notes:
 - x_q are exact integers in [-128,127] and w_t in {-1,0,1}; both are exact in
   bf16, so a bf16 matmul with fp32 PSUM accumulation reproduces the fp32
   reference einsum bit-for-bit (|sums| < 2^24).
 - round-half-to-even is done in fp32 via the magic constant 1.5*2^23.
 - ternary quantize sign(ws)*(|ws|>0.5) == RNE(clip(ws,-1,1)) exactly
   (ties at |ws|==0.5 round to 0 under RNE, matching the strict '>').
"""

import numpy as np

import concourse.bass as bass
import concourse.mybir as mybir
import concourse.tile as tile
from concourse import bacc
from concourse.bass_utils import run_bass_kernel_spmd
from concourse.masks import make_identity

F32 = mybir.dt.float32
BF16 = mybir.dt.bfloat16
ALU = mybir.AluOpType
AF = mybir.ActivationFunctionType

N_CORES = 8
R_FULL, K, N = 16384, 1024, 4096
R = R_FULL // N_CORES          # 2048 rows per core
RT = R // 128                  # 16 row tiles per core
KC = K // 128                  # 8 k-chunks
WS = N // 128                  # 32 weight strips (of 128 out-features)
NH = 2                         # n halves (2048 each)
NQ = 4                         # 512-wide psum tiles per half

C_MAGIC = 12582912.0           # 1.5 * 2^23: fp32 round-to-nearest-even trick
Q_EPS = 1e-5
NORM_EPS = 1e-6


def build_nc(g_is_ones: bool):
    nc = bacc.Bacc("TRN2", target_bir_lowering=False)

    x_d = nc.dram_tensor("x", [R, K], F32, kind="ExternalInput")
    w_d = nc.dram_tensor("w", [N, K], F32, kind="ExternalInput")
    if not g_is_ones:
        g_d = nc.dram_tensor("g", [1, K], F32, kind="ExternalInput")
    out_d = nc.dram_tensor("out", [R, N], F32, kind="ExternalOutput")

    with tile.TileContext(nc) as tc:
        with (
            tc.tile_pool(name="persist", bufs=1) as persist,
            tc.tile_pool(name="wst", bufs=3) as wst_pool,
            tc.tile_pool(name="wscr", bufs=2) as wscr_pool,
            tc.tile_pool(name="xp", bufs=3) as x_pool,
            tc.tile_pool(name="big", bufs=2) as big_pool,
            tc.tile_pool(name="stats", bufs=4) as st_pool,
            tc.tile_pool(name="osb", bufs=2) as osb_pool,
            tc.tile_pool(name="pmm", bufs=6, space="PSUM") as psum_mm,
            tc.tile_pool(name="ptp", bufs=2, space="PSUM") as psum_tp,
        ):
            # ---- constants ----
            ident = persist.tile([128, 128], BF16, tag="ident")
            make_identity(nc, ident[:])
            ones_col = persist.tile([128, 1], F32, tag="ones_col")
            nc.vector.memset(ones_col[:], 1.0)
            cb = persist.tile([128, 1], F32, tag="cb")
            nc.vector.memset(cb[:], C_MAGIC)

            if not g_is_ones:
                g_row = persist.tile([1, K], F32, tag="g_row")
                nc.sync.dma_start(g_row[:], g_d[:])
                g_b = persist.tile([128, K], F32, tag="g_b")
                nc.gpsimd.partition_broadcast(g_b[:], g_row[0:1, :])

            # w_t^T, bf16, split by n-half so matmuls only depend on the
            # strips of their half: wTT[h][:, j, n] = w_t[h*2048+n, j*128+kk]
            wTT = [
                persist.tile([128, KC, N // NH], BF16, tag=f"wTT{h}", name=f"wTT{h}")
                for h in range(NH)
            ]

            # ---- phase W1: stream w, accumulate per-strip |w| partials ----
            wpart = persist.tile([128, WS], F32, tag="wpart")
            with nc.named_scope("w_abs_sum"):
                for s in range(WS):
                    wst = wst_pool.tile([128, K], F32, tag="wst")
                    nc.sync.dma_start(wst[:], w_d[s * 128:(s + 1) * 128, :])
                    wabs = wscr_pool.tile([128, K], BF16, tag="wabs")
                    nc.scalar.activation(
                        wabs[:], wst[:], AF.Abs,
                        accum_out=wpart[:, s:s + 1],
                    )

                # total |w| sum: ones^T @ wpart -> [1, WS] -> reduce -> [1,1]
                ps_ws = psum_tp.tile([1, WS], F32, tag="tp")
                nc.tensor.matmul(ps_ws[:], lhsT=ones_col[:], rhs=wpart[:])
                wsum = st_pool.tile([1, 1], F32, tag="wsum")
                nc.vector.reduce_sum(wsum[:], ps_ws[:], axis=mybir.AxisListType.X)

                # w_scale = mean|w|; inv = 1/(w_scale + eps)
                wscale1 = st_pool.tile([1, 1], F32, tag="wscale1")
                nc.vector.tensor_scalar(
                    out=wscale1[:], in0=wsum[:], scalar1=1.0 / (N * K),
                    scalar2=None, op0=ALU.mult)
                speps1 = st_pool.tile([1, 1], F32, tag="speps1")
                nc.vector.tensor_scalar(
                    out=speps1[:], in0=wsum[:], scalar1=1.0 / (N * K),
                    scalar2=Q_EPS, op0=ALU.mult, op1=ALU.add)
                inv1 = st_pool.tile([1, 1], F32, tag="inv1")
                nc.vector.reciprocal(inv1[:], speps1[:])

                invb = persist.tile([128, 1], F32, tag="invb")
                nc.gpsimd.partition_broadcast(invb[:], inv1[0:1, :])
                wsb = persist.tile([128, 1], F32, tag="wsb")
                nc.gpsimd.partition_broadcast(wsb[:], wscale1[0:1, :])

            # ---- phase W2: ternarize + transpose ----
            with nc.named_scope("w_ternarize"):
                for s in range(WS):
                    wst2 = wst_pool.tile([128, K], F32, tag="wst2")
                    nc.sync.dma_start(wst2[:], w_d[s * 128:(s + 1) * 128, :])
                    # u = min(w * inv, 1); v = max(u, -1) + C  (fp32 RNE round)
                    u = wscr_pool.tile([128, K], F32, tag="wu")
                    nc.vector.tensor_scalar(
                        out=u[:], in0=wst2[:], scalar1=invb[:, 0:1],
                        scalar2=1.0, op0=ALU.mult, op1=ALU.min)
                    v = wscr_pool.tile([128, K], F32, tag="wv")
                    nc.vector.tensor_scalar(
                        out=v[:], in0=u[:], scalar1=-1.0,
                        scalar2=C_MAGIC, op0=ALU.max, op1=ALU.add)
                    wtn = wscr_pool.tile([128, K], BF16, tag="wtn")
                    nc.scalar.activation(wtn[:], v[:], AF.Copy, bias=-C_MAGIC)

                    h, hcol = s // (WS // NH), (s % (WS // NH)) * 128
                    for j in range(KC):
                        tp = psum_tp.tile([128, 128], BF16, tag="tp")
                        nc.tensor.transpose(
                            tp[:], wtn[:, j * 128:(j + 1) * 128], ident[:])
                        dst = wTT[h][:, j, hcol:hcol + 128]
                        if j < KC // 2:
                            nc.vector.tensor_copy(dst, tp[:])
                        else:
                            nc.scalar.copy(dst, tp[:])

            # ---- per row-tile: quantize x, transpose, matmul, rescale ----
            for rt in range(RT):
                with nc.named_scope("x_quant"):
                    xt = x_pool.tile([128, K], F32, tag="xt")
                    nc.sync.dma_start(xt[:], x_d[rt * 128:(rt + 1) * 128, :])

                    if g_is_ones:
                        xg = xt
                    else:
                        xg = big_pool.tile([128, K], F32, tag="xg")
                        nc.vector.tensor_mul(xg[:], xt[:], g_b[:])

                    # ssq = sum(x^2) (on raw x, pre-g); am = max|x*g|
                    xsq = big_pool.tile([128, K], F32, tag="xsq")
                    ssq = st_pool.tile([128, 1], F32, tag="ssq")
                    nc.vector.scalar_tensor_tensor(
                        out=xsq[:], in0=xt[:], scalar=1.0, in1=xt[:],
                        op0=ALU.mult, op1=ALU.mult, accum_out=ssq[:])
                    am = st_pool.tile([128, 1], F32, tag="am")
                    nc.vector.tensor_reduce(
                        am[:], xg[:], axis=mybir.AxisListType.X, op=ALU.max,
                        apply_absolute_value=True)

                    # rs = 1/sqrt(ms + eps) with one Newton step on sqrt
                    ms = st_pool.tile([128, 1], F32, tag="ms")
                    nc.vector.tensor_scalar(
                        out=ms[:], in0=ssq[:], scalar1=1.0 / K,
                        scalar2=NORM_EPS, op0=ALU.mult, op1=ALU.add)
                    s0 = st_pool.tile([128, 1], F32, tag="s0")
                    nc.scalar.sqrt(s0[:], ms[:])
                    r0 = st_pool.tile([128, 1], F32, tag="r0")
                    nc.vector.reciprocal(r0[:], s0[:])
                    t0 = st_pool.tile([128, 1], F32, tag="t0")
                    nc.vector.tensor_mul(t0[:], ms[:], r0[:])
                    t1 = st_pool.tile([128, 1], F32, tag="t1")
                    nc.vector.tensor_add(t1[:], t0[:], s0[:])
                    s1 = st_pool.tile([128, 1], F32, tag="s1")
                    nc.vector.tensor_scalar(
                        out=s1[:], in0=t1[:], scalar1=0.5,
                        scalar2=None, op0=ALU.mult)
                    rs = st_pool.tile([128, 1], F32, tag="rs")
                    nc.vector.reciprocal(rs[:], s1[:])

                    # x_scale = am*rs/127; srow = rs/(x_scale+eps); cs = x_scale*w_scale
                    axr = st_pool.tile([128, 1], F32, tag="axr")
                    nc.vector.tensor_mul(axr[:], am[:], rs[:])
                    xsc = st_pool.tile([128, 1], F32, tag="xsc")
                    nc.vector.tensor_scalar(
                        out=xsc[:], in0=axr[:], scalar1=1.0 / 127.0,
                        scalar2=None, op0=ALU.mult)
                    cs = st_pool.tile([128, 1], F32, tag="cs")
                    nc.vector.tensor_mul(cs[:], xsc[:], wsb[:])
                    sx = st_pool.tile([128, 1], F32, tag="sx")
                    nc.vector.tensor_scalar(
                        out=sx[:], in0=axr[:], scalar1=1.0 / 127.0,
                        scalar2=Q_EPS, op0=ALU.mult, op1=ALU.add)
                    dx = st_pool.tile([128, 1], F32, tag="dx")
                    nc.vector.reciprocal(dx[:], sx[:])
                    srow = st_pool.tile([128, 1], F32, tag="srow")
                    nc.vector.tensor_mul(srow[:], rs[:], dx[:])

                    # x_q = RNE(xg * srow) via +C (ACT) then -C (DVE, to bf16)
                    ux = big_pool.tile([128, K], F32, tag="ux")
                    nc.scalar.activation(
                        ux[:], xg[:], AF.Identity,
                        bias=cb[:, 0:1], scale=srow[:, 0:1])
                    xq = big_pool.tile([128, K], BF16, tag="xq")
                    nc.vector.tensor_scalar(
                        out=xq[:], in0=ux[:], scalar1=C_MAGIC,
                        scalar2=None, op0=ALU.subtract)

                    # transpose to [k, r]
                    xqT = x_pool.tile([128, KC, 128], BF16, tag="xqT")
                    for j in range(KC):
                        tpx = psum_tp.tile([128, 128], BF16, tag="tp")
                        nc.tensor.transpose(
                            tpx[:], xq[:, j * 128:(j + 1) * 128], ident[:])
                        if j < KC // 2:
                            nc.vector.tensor_copy(xqT[:, j, :], tpx[:])
                        else:
                            nc.scalar.copy(xqT[:, j, :], tpx[:])

                osb = osb_pool.tile([128, N], F32, tag="osb")
                for h in range(NH):
                    with nc.named_scope("mm"):
                        pst = [
                            psum_mm.tile([128, 512], F32, tag="pmm",
                                         name=f"pmm_{rt}_{h}_{_q}")
                            for _q in range(NQ)
                        ]
                        for j in range(KC):
                            for q in range(NQ):
                                nc.tensor.matmul(
                                    pst[q][:],
                                    lhsT=xqT[:, j, :],
                                    rhs=wTT[h][:, j, q * 512:(q + 1) * 512],
                                    start=(j == 0), stop=(j == KC - 1))
                    with nc.named_scope("out_scale"):
                        for q in range(NQ):
                            dst = osb[:, h * 2048 + q * 512:
                                      h * 2048 + (q + 1) * 512]
                            if q < NQ // 2:
                                nc.scalar.activation(
                                    dst, pst[q][:], AF.Copy, scale=cs[:, 0:1])
                            else:
                                nc.vector.tensor_scalar(
                                    out=dst, in0=pst[q][:], scalar1=cs[:, 0:1],
                                    scalar2=None, op0=ALU.mult)
                        nc.sync.dma_start(
                            out_d[rt * 128:(rt + 1) * 128,
                                  h * 2048:(h + 1) * 2048],
                            osb[:, h * 2048:(h + 1) * 2048])

    nc.compile()
    return nc


def _ensure_ntff_hook():
    """Make trace=True work: bass_utils imports antenv.axon_hooks, which is
    not present in this image. Shim it and install the ctypes-based NTFF
    profiling hook against libaxon_pjrt.so (same recipe as trn_boot)."""
    import sys
    import types
    try:
        import antenv.axon_hooks  # noqa: F401
        return
    except ImportError:
        pass
    mod = types.ModuleType("antenv.axon_hooks")
    mod._hook = None
    mod.set_axon_ntff_profile_hook = lambda h: setattr(mod, "_hook", h)
    mod.get_axon_ntff_profile_hook = lambda: mod._hook
    sys.modules["antenv.axon_hooks"] = mod
    import antenv
    antenv.axon_hooks = mod
    try:
        from trn_agent_boot.trn_boot import _ntff_profile_via_ctypes
        hook = _ntff_profile_via_ctypes("/opt/axon/libaxon_pjrt.so")
        if hook is not None:
            mod._hook = hook
    except Exception as e:  # degrade to no-trace
        print(f"ntff hook install failed: {e}")
    # no S3 in this sandbox; keep artifacts local
    import concourse.bass_utils as bu
    bu.upload_artifacts = lambda tmpdir: f"local://{tmpdir}"


_NC_CACHE = {}


def kernel(x: np.ndarray, weight: np.ndarray, norm_weight: np.ndarray) -> np.ndarray:
    import os
    x = np.ascontiguousarray(x, dtype=np.float32)
    weight = np.ascontiguousarray(weight, dtype=np.float32)
    norm_weight = np.ascontiguousarray(norm_weight, dtype=np.float32)

    B, S, Kin = x.shape
    xf = x.reshape(-1, Kin)
    g_is_ones = bool(np.all(norm_weight == 1.0))

    key = g_is_ones
    if key not in _NC_CACHE:
        _NC_CACHE[key] = build_nc(g_is_ones)
    nc = _NC_CACHE[key]

    in_maps = []
    for i in range(N_CORES):
        m = {"x": xf[i * R:(i + 1) * R], "w": weight}
        if not g_is_ones:
            m["g"] = norm_weight.reshape(1, Kin)
        in_maps.append(m)

    trace = bool(int(os.environ.get("BITLIN_TRACE", "0")))
    if trace:
        _ensure_ntff_hook()
    res = run_bass_kernel_spmd(
        nc, in_maps, core_ids=list(range(N_CORES)), trace=trace,
    )
    if trace:
        kernel.last_results = res
    out = np.concatenate([r["out"] for r in res.results], axis=0)
    return out.reshape(B, S, weight.shape[0]).astype(np.float32)
